# revision 4
# baseline (speedup 1.0000x reference)
"""Trainium2 Bass kernel for nn_EntityEncoder (gnn_message_passing).

Full inputs in, full outputs out. Internally: data-parallel over batch across
8 NeuronCores (128 batch rows per core). Embedding rows are fetched on-device
via dma_gather from per-core compacted tables (int16 index constraint);
attention scores via fused DVE dot-products; attention-apply via per-batch
stationary matmuls on the tensor engine; residual + LayerNorm fused on
DVE/ACT.
"""

import numpy as np

import concourse.tile_sem_assignment as _tsa

# Walrus rejects instructions carrying >2 semaphore waits and Tile's
# FIFO-dominance wait elision is disabled; a single SWDGE completion lane
# keeps every instruction's wait count within the ISA limit.
_tsa.NUM_SWDGE_GLOBAL_SEMS = 1

from concourse import bacc, bass, mybir  # noqa: E402
import concourse.tile as tile  # noqa: E402
from concourse.bass_utils import run_bass_kernel_spmd  # noqa: E402
from concourse.masks import make_identity  # noqa: E402

# Problem constants (hardcoded per harness contract).
D = 128            # embed dim
B_FULL = 1024      # full batch
M = 200            # max neighbors
N_CORES = 8
B = B_FULL // N_CORES  # 128 rows per core
PAD_IDX = 100000
LN_EPS = 1e-5

# Compact-table sizing: per side <=128*200 unique rel/tail ids, +1 zero row.
U_MAX = 25728      # fixed table row count (>= 25601), identical across cores
UH_MAX = 257       # head table rows (<=256 unique entity ids + zero row)

REL_CHUNK_COLS = 50   # m-values per rel gather  -> 6400 rows/instr
TAIL_CHUNK_COLS = 32  # tile columns per tail gather -> 4096 rows/instr

_F32 = mybir.dt.float32
_I16 = mybir.dt.int16
_I32 = mybir.dt.int32
_AX = mybir.AxisListType
_OP = mybir.AluOpType
_ACT = mybir.ActivationFunctionType

_PROGRAM_CACHE = {}


def _wrap16(ids16: np.ndarray) -> np.ndarray:
    """Flat int16 index list -> [128, N/16] wrapped/replicated dma_gather layout."""
    n = ids16.shape[0]
    assert n % 16 == 0
    blk = ids16.reshape(n // 16, 16).T  # [16, n/16]
    return np.tile(blk, (8, 1)).astype(np.int16)


def _build_side(nc, tc, consts, side, ios):
    """Emit one side's (left/right) compute. consts holds shared tiles."""
    sb = consts["sb"]
    relbuf = consts["relbuf"]
    tailbuf = consts["tailbuf"]
    psum = consts["psum"]
    u_s = consts[f"u_{side}"]
    headT_s = consts[f"headT_{side}"]
    head_nat_s = consts[f"head_nat_{side}"]

    rel_table = ios[f"rel_table_{side}"]
    tail_table = ios[f"tail_table_{side}"]
    rel_idx_d = ios[f"rel_idx_{side}"]
    tail_idx_d = ios[f"tail_idx_{side}"]
    pen_d = ios[f"pen_{side}"]
    out_d = ios[f"out_{side}"]

    # --- small loads -------------------------------------------------------
    rel_idx = sb.tile([128, (M * 128) // 16], _I16, tag=f"rel_idx_{side}")
    nc.gpsimd.dma_start(out=rel_idx[:], in_=rel_idx_d[:])
    tail_idx = sb.tile([128, (256 * 128) // 16], _I16, tag=f"tail_idx_{side}")
    nc.gpsimd.dma_start(out=tail_idx[:], in_=tail_idx_d[:])
    pen = sb.tile([128, M], _F32, tag=f"pen_{side}")
    nc.gpsimd.dma_start(out=pen[:], in_=pen_d[:])

    # --- scores: score[b, m] = u[b, :] . rel[b, m, :]  ---------------------
    score = sb.tile([128, M], _F32, tag=f"score_{side}")
    for mc in range(0, M, REL_CHUNK_COLS):
        k = min(REL_CHUNK_COLS, M - mc)
        rel_chunk = relbuf.tile([128, k, D], _F32, tag="rel_chunk")
        nc.gpsimd.dma_gather(
            rel_chunk[:],
            rel_table[:],
            rel_idx[:, (mc * 128) // 16 : ((mc + k) * 128) // 16],
            k * 128,
            k * 128,
            D,
            single_packet=False,
        )
        for j in range(k):
            scratch = consts["scratch"].tile([128, D], _F32, tag="dot_scratch")
            nc.vector.scalar_tensor_tensor(
                out=scratch[:],
                in0=rel_chunk[:, j, :],
                scalar=1.0,
                in1=u_s[:],
                op0=_OP.mult,
                op1=_OP.mult,
                accum_out=score[:, mc + j : mc + j + 1],
            )

    # mask penalty (pad neighbors -> -1e30)
    nc.vector.tensor_tensor(out=score[:], in0=score[:], in1=pen[:], op=_OP.add)

    # --- softmax over m ----------------------------------------------------
    rmax = sb.tile([128, 1], _F32, tag=f"rmax_{side}")
    nc.vector.reduce_max(rmax[:], score[:], axis=_AX.X)
    negmax = sb.tile([128, 1], _F32, tag=f"negmax_{side}")
    nc.vector.tensor_scalar_mul(negmax[:], rmax[:], -1.0)
    expt = sb.tile([128, M], _F32, tag=f"expt_{side}")
    zsum = sb.tile([128, 1], _F32, tag=f"zsum_{side}")
    nc.scalar.activation(
        out=expt[:], in_=score[:], func=_ACT.Exp,
        bias=negmax[:, :1], scale=1.0, accum_out=zsum[:],
    )
    rz = sb.tile([128, 1], _F32, tag=f"rz_{side}")
    nc.vector.reciprocal(rz[:], zsum[:])
    att = sb.tile([128, M], _F32, tag=f"att_{side}")
    nc.vector.tensor_scalar_mul(att[:], expt[:], rz[:, :1])

    # --- transpose att -> [m, b] columns for per-b matmul rhs --------------
    attT0_p = psum.tile([128, 128], _F32, space="PSUM", tag="tr_p")
    nc.tensor.transpose(out=attT0_p[:], in_=att[:, 0:128], identity=consts["ident"][:])
    attT0 = sb.tile([128, 128], _F32, tag=f"attT0_{side}")
    nc.vector.tensor_copy(out=attT0[:], in_=attT0_p[:])

    attT1_p = psum.tile([72, 128], _F32, space="PSUM", tag="tr_p")
    nc.tensor.transpose(out=attT1_p[:], in_=att[:, 128:200], identity=consts["ident"][:])
    attT1 = sb.tile([72, 128], _F32, tag=f"attT1_{side}")
    nc.vector.tensor_copy(out=attT1[:], in_=attT1_p[:])

    # --- attention-apply: aggT[:, b] = sum_m att[b, m] * tail[b, m, :] -----
    aggT_p = psum.tile([128, 128], _F32, space="PSUM", tag="aggT_p")
    for tc0 in range(0, 256, TAIL_CHUNK_COLS):
        k = TAIL_CHUNK_COLS
        tail_chunk = tailbuf.tile([128, k, D], _F32, tag="tail_chunk")
        nc.gpsimd.dma_gather(
            tail_chunk[:],
            tail_table[:],
            tail_idx[:, (tc0 * 128) // 16 : ((tc0 + k) * 128) // 16],
            k * 128,
            k * 128,
            D,
            single_packet=False,
        )
        for j in range(0, k, 2):
            b = (tc0 + j) // 2
            nc.tensor.matmul(
                out=aggT_p[:, b : b + 1],
                lhsT=tail_chunk[:, j, :],
                rhs=attT0[:, b : b + 1],
                start=True, stop=False,
            )
            nc.tensor.matmul(
                out=aggT_p[:, b : b + 1],
                lhsT=tail_chunk[:72, j + 1, :],
                rhs=attT1[:72, b : b + 1],
                start=False, stop=True,
            )
    aggT = sb.tile([128, 128], _F32, tag=f"aggT_{side}")
    nc.vector.tensor_copy(out=aggT[:], in_=aggT_p[:])

    # --- branch: h = relu(agg @ Wt^T + head @ Wh^T);  x = h + head; LN -----
    h_p = consts["psum1"].tile([128, 128], _F32, space="PSUM", tag="h_p")
    nc.tensor.matmul(out=h_p[:], lhsT=aggT[:], rhs=consts["W_tailT"][:],
                     start=True, stop=False)
    nc.tensor.matmul(out=h_p[:], lhsT=headT_s[:], rhs=consts["W_headT"][:],
                     start=False, stop=True)
    h = sb.tile([128, 128], _F32, tag=f"h_{side}")
    nc.scalar.activation(out=h[:], in_=h_p[:], func=_ACT.Relu)

    x = sb.tile([128, 128], _F32, tag=f"x_{side}")
    nc.vector.tensor_tensor(out=x[:], in0=h[:], in1=head_nat_s[:], op=_OP.add)

    s1 = sb.tile([128, 1], _F32, tag=f"s1_{side}")
    nc.vector.reduce_sum(s1[:], x[:], axis=_AX.X)
    negmu = sb.tile([128, 1], _F32, tag=f"negmu_{side}")
    nc.vector.tensor_scalar_mul(negmu[:], s1[:], -1.0 / D)
    xc = sb.tile([128, 128], _F32, tag=f"xc_{side}")
    nc.scalar.activation(out=xc[:], in_=x[:], func=_ACT.Identity, bias=negmu[:, :1])
    sq = sb.tile([128, 128], _F32, tag=f"sq_{side}")
    ssq = sb.tile([128, 1], _F32, tag=f"ssq_{side}")
    nc.scalar.activation(out=sq[:], in_=xc[:], func=_ACT.Square, accum_out=ssq[:])
    std = sb.tile([128, 1], _F32, tag=f"std_{side}")
    # std = sqrt(var + eps) = sqrt(ssq/D + eps)
    nc.scalar.activation(out=std[:], in_=ssq[:], func=_ACT.Sqrt,
                         bias=consts["eps"][:, :1], scale=1.0 / D)
    rstd = sb.tile([128, 1], _F32, tag=f"rstd_{side}")
    nc.vector.reciprocal(rstd[:], std[:])

    y = sb.tile([128, 128], _F32, tag=f"y_{side}")
    nc.vector.scalar_tensor_tensor(
        out=y[:], in0=xc[:], scalar=rstd[:, :1], in1=consts["gamma_b"][:],
        op0=_OP.mult, op1=_OP.mult,
    )
    yb = sb.tile([128, 128], _F32, tag=f"yb_{side}")
    nc.vector.tensor_tensor(out=yb[:], in0=y[:], in1=consts["beta_b"][:], op=_OP.add)
    nc.gpsimd.dma_start(out=out_d[:], in_=yb[:])


def _build_program():
    nc = bacc.Bacc(None, target_bir_lowering=False, debug=False)

    ios = {}
    for side in ("L", "R"):
        ios[f"rel_table_{side}"] = nc.declare_dram_parameter(
            f"rel_table_{side}", [U_MAX, D], _F32, isOutput=False)
        ios[f"tail_table_{side}"] = nc.declare_dram_parameter(
            f"tail_table_{side}", [U_MAX, D], _F32, isOutput=False)
        ios[f"rel_idx_{side}"] = nc.declare_dram_parameter(
            f"rel_idx_{side}", [128, (M * 128) // 16], _I16, isOutput=False)
        ios[f"tail_idx_{side}"] = nc.declare_dram_parameter(
            f"tail_idx_{side}", [128, (256 * 128) // 16], _I16, isOutput=False)
        ios[f"pen_{side}"] = nc.declare_dram_parameter(
            f"pen_{side}", [128, M], _F32, isOutput=False)
        ios[f"out_{side}"] = nc.declare_dram_parameter(
            f"out_{side}", [128, D], _F32, isOutput=True)
    ios["head_table"] = nc.declare_dram_parameter(
        "head_table", [UH_MAX, D], _F32, isOutput=False)
    ios["ent_idx"] = nc.declare_dram_parameter(
        "ent_idx", [128, 2], _I32, isOutput=False)
    for w in ("W_bil", "W_tailT", "W_headT", "gamma_b", "beta_b"):
        ios[w] = nc.declare_dram_parameter(w, [128, 128], _F32, isOutput=False)

    with tile.TileContext(nc) as tc:
        with (
            tc.tile_pool(name="sb", bufs=1) as sb,
            tc.tile_pool(name="relbuf", bufs=2) as relbuf,
            tc.tile_pool(name="tailbuf", bufs=2) as tailbuf,
            tc.tile_pool(name="scratch", bufs=4) as scratch,
            tc.tile_pool(name="psum", bufs=2, space="PSUM") as psum,
            tc.tile_pool(name="psum1", bufs=1, space="PSUM") as psum1,
        ):
            consts = {
                "sb": sb, "relbuf": relbuf, "tailbuf": tailbuf,
                "scratch": scratch, "psum": psum, "psum1": psum1,
            }
            # constants
            for w in ("W_bil", "W_tailT", "W_headT", "gamma_b", "beta_b"):
                t = sb.tile([128, 128], _F32, tag=w)
                nc.gpsimd.dma_start(out=t[:], in_=ios[w][:])
                consts[w] = t
            ident = sb.tile([128, 128], _F32, tag="ident")
            make_identity(nc, ident[:])
            consts["ident"] = ident
            eps = sb.tile([128, 1], _F32, tag="eps")
            nc.vector.memset(eps[:], LN_EPS)
            consts["eps"] = eps

            # heads: gather, transpose, u = (headR - headL) @ W_bil
            ent_idx = sb.tile([128, 2], _I32, tag="ent_idx")
            nc.gpsimd.dma_start(out=ent_idx[:], in_=ios["ent_idx"][:])
            head_nat = {}
            headT = {}
            for i, side in enumerate(("L", "R")):
                hn = sb.tile([128, D], _F32, tag=f"head_nat_{side}")
                nc.gpsimd.indirect_dma_start(
                    out=hn[:], out_offset=None, in_=ios["head_table"][:],
                    in_offset=bass.IndirectOffsetOnAxis(
                        ap=ent_idx[:, i : i + 1], axis=0),
                )
                head_nat[side] = hn
                consts[f"head_nat_{side}"] = hn
                hT_p = psum.tile([128, 128], _F32, space="PSUM", tag="tr_p")
                nc.tensor.transpose(out=hT_p[:], in_=hn[:], identity=ident[:])
                hT = sb.tile([128, 128], _F32, tag=f"headT_{side}")
                nc.vector.tensor_copy(out=hT[:], in_=hT_p[:])
                headT[side] = hT
                consts[f"headT_{side}"] = hT

            wrT = sb.tile([128, 128], _F32, tag="wrT")
            nc.vector.tensor_tensor(
                out=wrT[:], in0=headT["R"][:], in1=headT["L"][:], op=_OP.subtract)
            # u[b, e] = sum_d wrT[d, b] * W_bil[d, e]   (for both sides)
            u_p = psum1.tile([128, 128], _F32, space="PSUM", tag="u_p")
            nc.tensor.matmul(out=u_p[:], lhsT=wrT[:], rhs=consts["W_bil"][:],
                             start=True, stop=True)
            u = sb.tile([128, 128], _F32, tag="u")
            nc.vector.tensor_copy(out=u[:], in_=u_p[:])
            consts["u_L"] = u
            consts["u_R"] = u

            for side in ("L", "R"):
                _build_side(nc, tc, consts, side, ios)

    nc.finalize()
    return nc


def _prep_inputs(entity, conn_left, conn_right, emb, W_bil, W_tail, W_head,
                 gamma, beta):
    """Host-side sharding + compaction. Returns per-core input maps."""
    entity = np.asarray(entity).astype(np.int32)
    conn_left = np.asarray(conn_left).astype(np.int32)
    conn_right = np.asarray(conn_right).astype(np.int32)
    emb = np.ascontiguousarray(np.asarray(emb), dtype=np.float32)
    W_bil = np.asarray(W_bil, dtype=np.float32)
    W_tailT = np.ascontiguousarray(np.asarray(W_tail, dtype=np.float32).T)
    W_headT = np.ascontiguousarray(np.asarray(W_head, dtype=np.float32).T)
    gamma_b = np.ascontiguousarray(
        np.broadcast_to(np.asarray(gamma, np.float32), (128, D)))
    beta_b = np.ascontiguousarray(
        np.broadcast_to(np.asarray(beta, np.float32), (128, D)))

    in_maps = []
    for c in range(N_CORES):
        sl = slice(c * B, (c + 1) * B)
        ent = entity[sl]                       # [128, 2]
        m = {
            "W_bil": W_bil, "W_tailT": W_tailT, "W_headT": W_headT,
            "gamma_b": gamma_b, "beta_b": beta_b,
        }
        # heads
        uniq_h, inv_h = np.unique(ent, return_inverse=True)
        head_table = np.zeros((UH_MAX, D), np.float32)
        head_table[: uniq_h.shape[0]] = emb[uniq_h]
        m["head_table"] = head_table
        m["ent_idx"] = inv_h.reshape(128, 2).astype(np.int32)

        for side, conn in (("L", conn_left), ("R", conn_right)):
            ids = conn[sl]                     # [128, 200, 2]
            rel_ids, tail_ids = ids[..., 0], ids[..., 1]

            uniq_r, inv_r = np.unique(rel_ids, return_inverse=True)
            inv_r = inv_r.reshape(B, M)
            rel_table = np.zeros((U_MAX, D), np.float32)
            rel_table[: uniq_r.shape[0]] = emb[uniq_r]
            m[f"rel_table_{side}"] = rel_table
            # m-grouped: position i = m*128 + b
            m[f"rel_idx_{side}"] = _wrap16(
                inv_r.T.reshape(-1).astype(np.int16))

            uniq_t, inv_t = np.unique(tail_ids, return_inverse=True)
            inv_t = inv_t.reshape(B, M)
            ut = uniq_t.shape[0]
            tail_table = np.zeros((U_MAX, D), np.float32)
            tail_table[:ut] = emb[uniq_t]
            m[f"tail_table_{side}"] = tail_table
            # b-grouped padded: X[b, c, p] = inv_t[b, c*128+p] (pad -> ut)
            X = np.full((B, 2, 128), ut, np.int16)
            X[:, 0, :] = inv_t[:, 0:128]
            X[:, 1, :72] = inv_t[:, 128:200]
            m[f"tail_idx_{side}"] = _wrap16(X.reshape(-1))

            m[f"pen_{side}"] = np.where(
                rel_ids == PAD_IDX, -1e30, 0.0).astype(np.float32)
        in_maps.append(m)
    return in_maps


def _get_program():
    if "nc" not in _PROGRAM_CACHE:
        _PROGRAM_CACHE["nc"] = _build_program()
    return _PROGRAM_CACHE["nc"]


def kernel(entity, conn_left, conn_right, emb, W_bil, W_tail, W_head,
           gamma, beta):
    nc = _get_program()
    in_maps = _prep_inputs(entity, conn_left, conn_right, emb, W_bil, W_tail,
                           W_head, gamma, beta)
    res = run_bass_kernel_spmd(nc, in_maps, core_ids=list(range(N_CORES)))
    left = np.concatenate([np.asarray(r["out_L"]) for r in res.results], axis=0)
    right = np.concatenate([np.asarray(r["out_R"]) for r in res.results], axis=0)
    return left, right


# revision 7
# speedup vs baseline: 29.2147x; 29.2147x over previous
"""Trainium2 Bass kernel for nn_EntityEncoder (gnn_message_passing).

Full inputs in, full outputs out. Internally: data-parallel over batch across
8 NeuronCores (128 batch rows per core). Embedding rows are fetched on-device
via dma_gather from per-core compacted tables (int16 index constraint);
attention scores via fused DVE dot-products; attention-apply via per-batch
stationary matmuls on the tensor engine; residual + LayerNorm fused on
DVE/ACT.
"""

import numpy as np

import concourse.tile_sem_assignment as _tsa

# Walrus rejects instructions carrying >2 semaphore waits and Tile's
# FIFO-dominance wait elision is disabled; a single SWDGE completion lane
# keeps every instruction's wait count within the ISA limit.
_tsa.NUM_SWDGE_GLOBAL_SEMS = 1

from concourse import bacc, bass, mybir  # noqa: E402
import concourse.tile as tile  # noqa: E402
from concourse.bass_utils import run_bass_kernel_spmd  # noqa: E402
from concourse.masks import make_identity  # noqa: E402

# Problem constants (hardcoded per harness contract).
D = 128            # embed dim
B_FULL = 1024      # full batch
M = 200            # max neighbors
N_CORES = 8
B = B_FULL // N_CORES  # 128 rows per core
PAD_IDX = 100000
LN_EPS = 1e-5

# Compact-table sizing: per side <=128*200 unique rel/tail ids, +1 zero row.
U_MAX = 25728      # fixed table row count (>= 25601), identical across cores
UH_MAX = 257       # head table rows (<=256 unique entity ids + zero row)

REL_CHUNK_COLS = 50   # m-values per rel gather  -> 6400 rows/instr
TAIL_CHUNK_COLS = 32  # tile columns per tail gather -> 4096 rows/instr

_F32 = mybir.dt.float32
_I16 = mybir.dt.int16
_I32 = mybir.dt.int32
_AX = mybir.AxisListType
_OP = mybir.AluOpType
_ACT = mybir.ActivationFunctionType

_PROGRAM_CACHE = {}


def _wrap16(ids16: np.ndarray) -> np.ndarray:
    """Flat int16 index list -> [128, N/16] wrapped/replicated dma_gather layout."""
    n = ids16.shape[0]
    assert n % 16 == 0
    blk = ids16.reshape(n // 16, 16).T  # [16, n/16]
    return np.tile(blk, (8, 1)).astype(np.int16)


def _build_side(nc, tc, consts, side, ios):
    """Emit one side's (left/right) compute. consts holds shared tiles."""
    sb = consts["sb"]
    relbuf = consts["relbuf"]
    tailbuf = consts["tailbuf"]
    psum = consts["psum"]
    u_s = consts[f"u_{side}"]
    headT_s = consts[f"headT_{side}"]
    head_nat_s = consts[f"head_nat_{side}"]

    rel_table = ios[f"rel_table_{side}"]
    tail_table = ios[f"tail_table_{side}"]
    rel_idx_d = ios[f"rel_idx_{side}"]
    tail_idx_d = ios[f"tail_idx_{side}"]
    pen_d = ios[f"pen_{side}"]
    out_d = ios[f"out_{side}"]

    # --- small loads -------------------------------------------------------
    rel_idx = sb.tile([128, (M * 128) // 16], _I16, tag=f"rel_idx_{side}")
    nc.gpsimd.dma_start(out=rel_idx[:], in_=rel_idx_d[:])
    tail_idx = sb.tile([128, (256 * 128) // 16], _I16, tag=f"tail_idx_{side}")
    nc.gpsimd.dma_start(out=tail_idx[:], in_=tail_idx_d[:])
    pen = sb.tile([128, M], _F32, tag=f"pen_{side}")
    nc.gpsimd.dma_start(out=pen[:], in_=pen_d[:])

    # --- scores: score[b, m] = u[b, :] . rel[b, m, :]  ---------------------
    score = sb.tile([128, M], _F32, tag=f"score_{side}")
    for mc in range(0, M, REL_CHUNK_COLS):
        k = min(REL_CHUNK_COLS, M - mc)
        rel_chunk = relbuf.tile([128, k, D], _F32, tag="rel_chunk")
        nc.gpsimd.dma_gather(
            rel_chunk[:],
            rel_table[:],
            rel_idx[:, (mc * 128) // 16 : ((mc + k) * 128) // 16],
            k * 128,
            k * 128,
            D,
            single_packet=False,
        )
        for j in range(k):
            scratch = consts["scratch"].tile([128, D], _F32, tag="dot_scratch")
            nc.vector.scalar_tensor_tensor(
                out=scratch[:],
                in0=rel_chunk[:, j, :],
                scalar=1.0,
                in1=u_s[:],
                op0=_OP.mult,
                op1=_OP.mult,
                accum_out=score[:, mc + j : mc + j + 1],
            )

    # mask penalty (pad neighbors -> -1e30)
    nc.vector.tensor_tensor(out=score[:], in0=score[:], in1=pen[:], op=_OP.add)

    # --- softmax over m ----------------------------------------------------
    rmax = sb.tile([128, 1], _F32, tag=f"rmax_{side}")
    nc.vector.reduce_max(rmax[:], score[:], axis=_AX.X)
    negmax = sb.tile([128, 1], _F32, tag=f"negmax_{side}")
    nc.vector.tensor_scalar_mul(negmax[:], rmax[:], -1.0)
    expt = sb.tile([128, M], _F32, tag=f"expt_{side}")
    zsum = sb.tile([128, 1], _F32, tag=f"zsum_{side}")
    nc.scalar.activation(
        out=expt[:], in_=score[:], func=_ACT.Exp,
        bias=negmax[:, :1], scale=1.0, accum_out=zsum[:],
    )
    rz = sb.tile([128, 1], _F32, tag=f"rz_{side}")
    nc.vector.reciprocal(rz[:], zsum[:])
    att = sb.tile([128, M], _F32, tag=f"att_{side}")
    nc.vector.tensor_scalar_mul(att[:], expt[:], rz[:, :1])

    # --- transpose att -> [m, b] columns for per-b matmul rhs --------------
    attT0_p = psum.tile([128, 128], _F32, space="PSUM", tag="tr_p")
    nc.tensor.transpose(out=attT0_p[:], in_=att[:, 0:128], identity=consts["ident"][:])
    attT0 = sb.tile([128, 128], _F32, tag=f"attT0_{side}")
    nc.vector.tensor_copy(out=attT0[:], in_=attT0_p[:])

    attT1_p = psum.tile([72, 128], _F32, space="PSUM", tag="tr_p")
    nc.tensor.transpose(out=attT1_p[:], in_=att[:, 128:200], identity=consts["ident"][:])
    attT1 = sb.tile([72, 128], _F32, tag=f"attT1_{side}")
    nc.vector.tensor_copy(out=attT1[:], in_=attT1_p[:])

    # --- attention-apply: aggT[:, b] = sum_m att[b, m] * tail[b, m, :] -----
    aggT_p = psum.tile([128, 128], _F32, space="PSUM", tag="aggT_p")
    for tc0 in range(0, 256, TAIL_CHUNK_COLS):
        k = TAIL_CHUNK_COLS
        tail_chunk = tailbuf.tile([128, k, D], _F32, tag="tail_chunk")
        nc.gpsimd.dma_gather(
            tail_chunk[:],
            tail_table[:],
            tail_idx[:, (tc0 * 128) // 16 : ((tc0 + k) * 128) // 16],
            k * 128,
            k * 128,
            D,
            single_packet=False,
        )
        for j in range(0, k, 2):
            b = (tc0 + j) // 2
            nc.tensor.matmul(
                out=aggT_p[:, b : b + 1],
                lhsT=tail_chunk[:, j, :],
                rhs=attT0[:, b : b + 1],
                start=True, stop=False,
            )
            nc.tensor.matmul(
                out=aggT_p[:, b : b + 1],
                lhsT=tail_chunk[:72, j + 1, :],
                rhs=attT1[:72, b : b + 1],
                start=False, stop=True,
            )
    aggT = sb.tile([128, 128], _F32, tag=f"aggT_{side}")
    nc.vector.tensor_copy(out=aggT[:], in_=aggT_p[:])

    # --- branch: h = relu(agg @ Wt^T + head @ Wh^T);  x = h + head; LN -----
    h_p = consts["psum1"].tile([128, 128], _F32, space="PSUM", tag="h_p")
    nc.tensor.matmul(out=h_p[:], lhsT=aggT[:], rhs=consts["W_tailT"][:],
                     start=True, stop=False)
    nc.tensor.matmul(out=h_p[:], lhsT=headT_s[:], rhs=consts["W_headT"][:],
                     start=False, stop=True)
    h = sb.tile([128, 128], _F32, tag=f"h_{side}")
    nc.scalar.activation(out=h[:], in_=h_p[:], func=_ACT.Relu)

    x = sb.tile([128, 128], _F32, tag=f"x_{side}")
    nc.vector.tensor_tensor(out=x[:], in0=h[:], in1=head_nat_s[:], op=_OP.add)

    s1 = sb.tile([128, 1], _F32, tag=f"s1_{side}")
    nc.vector.reduce_sum(s1[:], x[:], axis=_AX.X)
    negmu = sb.tile([128, 1], _F32, tag=f"negmu_{side}")
    nc.vector.tensor_scalar_mul(negmu[:], s1[:], -1.0 / D)
    xc = sb.tile([128, 128], _F32, tag=f"xc_{side}")
    nc.scalar.activation(out=xc[:], in_=x[:], func=_ACT.Identity, bias=negmu[:, :1])
    sq = sb.tile([128, 128], _F32, tag=f"sq_{side}")
    ssq = sb.tile([128, 1], _F32, tag=f"ssq_{side}")
    nc.scalar.activation(out=sq[:], in_=xc[:], func=_ACT.Square, accum_out=ssq[:])
    std = sb.tile([128, 1], _F32, tag=f"std_{side}")
    # std = sqrt(var + eps) = sqrt(ssq/D + eps)
    nc.scalar.activation(out=std[:], in_=ssq[:], func=_ACT.Sqrt,
                         bias=consts["eps"][:, :1], scale=1.0 / D)
    rstd = sb.tile([128, 1], _F32, tag=f"rstd_{side}")
    nc.vector.reciprocal(rstd[:], std[:])

    y = sb.tile([128, 128], _F32, tag=f"y_{side}")
    nc.vector.scalar_tensor_tensor(
        out=y[:], in0=xc[:], scalar=rstd[:, :1], in1=consts["gamma_b"][:],
        op0=_OP.mult, op1=_OP.mult,
    )
    yb = sb.tile([128, 128], _F32, tag=f"yb_{side}")
    nc.vector.tensor_tensor(out=yb[:], in0=y[:], in1=consts["beta_b"][:], op=_OP.add)
    nc.gpsimd.dma_start(out=out_d[:], in_=yb[:])


def _build_program(repeat: int = 1):
    nc = bacc.Bacc(None, target_bir_lowering=False, debug=False)

    ios = {}
    for side in ("L", "R"):
        ios[f"rel_table_{side}"] = nc.declare_dram_parameter(
            f"rel_table_{side}", [U_MAX, D], _F32, isOutput=False)
        ios[f"tail_table_{side}"] = nc.declare_dram_parameter(
            f"tail_table_{side}", [U_MAX, D], _F32, isOutput=False)
        ios[f"rel_idx_{side}"] = nc.declare_dram_parameter(
            f"rel_idx_{side}", [128, (M * 128) // 16], _I16, isOutput=False)
        ios[f"tail_idx_{side}"] = nc.declare_dram_parameter(
            f"tail_idx_{side}", [128, (256 * 128) // 16], _I16, isOutput=False)
        ios[f"pen_{side}"] = nc.declare_dram_parameter(
            f"pen_{side}", [128, M], _F32, isOutput=False)
        ios[f"out_{side}"] = nc.declare_dram_parameter(
            f"out_{side}", [128, D], _F32, isOutput=True)
    ios["head_table"] = nc.declare_dram_parameter(
        "head_table", [UH_MAX, D], _F32, isOutput=False)
    ios["ent_idx"] = nc.declare_dram_parameter(
        "ent_idx", [128, 2], _I32, isOutput=False)
    for w in ("W_bil", "W_tailT", "W_headT", "gamma_b", "beta_b"):
        ios[w] = nc.declare_dram_parameter(w, [128, 128], _F32, isOutput=False)

    with tile.TileContext(nc) as tc:
        with (
            tc.tile_pool(name="sb", bufs=1) as sb,
            tc.tile_pool(name="relbuf", bufs=2) as relbuf,
            tc.tile_pool(name="tailbuf", bufs=2) as tailbuf,
            tc.tile_pool(name="scratch", bufs=4) as scratch,
            tc.tile_pool(name="psum", bufs=2, space="PSUM") as psum,
            tc.tile_pool(name="psum1", bufs=1, space="PSUM") as psum1,
        ):
            consts = {
                "sb": sb, "relbuf": relbuf, "tailbuf": tailbuf,
                "scratch": scratch, "psum": psum, "psum1": psum1,
            }
            # constants
            for w in ("W_bil", "W_tailT", "W_headT", "gamma_b", "beta_b"):
                t = sb.tile([128, 128], _F32, tag=w)
                nc.gpsimd.dma_start(out=t[:], in_=ios[w][:])
                consts[w] = t
            ident = sb.tile([128, 128], _F32, tag="ident")
            make_identity(nc, ident[:])
            consts["ident"] = ident
            eps = sb.tile([128, 1], _F32, tag="eps")
            nc.vector.memset(eps[:], LN_EPS)
            consts["eps"] = eps

            def body():
                # heads: gather, transpose, u = (headR - headL) @ W_bil
                ent_idx = sb.tile([128, 2], _I32, tag="ent_idx")
                nc.gpsimd.dma_start(out=ent_idx[:], in_=ios["ent_idx"][:])
                headT = {}
                for i, side in enumerate(("L", "R")):
                    hn = sb.tile([128, D], _F32, tag=f"head_nat_{side}")
                    nc.gpsimd.indirect_dma_start(
                        out=hn[:], out_offset=None, in_=ios["head_table"][:],
                        in_offset=bass.IndirectOffsetOnAxis(
                            ap=ent_idx[:, i : i + 1], axis=0),
                    )
                    consts[f"head_nat_{side}"] = hn
                    hT_p = psum.tile([128, 128], _F32, space="PSUM", tag="tr_p")
                    nc.tensor.transpose(out=hT_p[:], in_=hn[:], identity=ident[:])
                    hT = sb.tile([128, 128], _F32, tag=f"headT_{side}")
                    nc.vector.tensor_copy(out=hT[:], in_=hT_p[:])
                    headT[side] = hT
                    consts[f"headT_{side}"] = hT

                wrT = sb.tile([128, 128], _F32, tag="wrT")
                nc.vector.tensor_tensor(
                    out=wrT[:], in0=headT["R"][:], in1=headT["L"][:],
                    op=_OP.subtract)
                # u[b, e] = sum_d wrT[d, b] * W_bil[d, e]   (for both sides)
                u_p = psum1.tile([128, 128], _F32, space="PSUM", tag="u_p")
                nc.tensor.matmul(out=u_p[:], lhsT=wrT[:], rhs=consts["W_bil"][:],
                                 start=True, stop=True)
                u = sb.tile([128, 128], _F32, tag="u")
                nc.vector.tensor_copy(out=u[:], in_=u_p[:])
                consts["u_L"] = u
                consts["u_R"] = u

                for side in ("L", "R"):
                    _build_side(nc, tc, consts, side, ios)

            if repeat == 1:
                body()
            else:
                with tc.For_i(0, repeat, 1):
                    body()

    nc.finalize()
    return nc


def _prep_inputs(entity, conn_left, conn_right, emb, W_bil, W_tail, W_head,
                 gamma, beta):
    """Host-side sharding + compaction. Returns per-core input maps."""
    entity = np.asarray(entity).astype(np.int32)
    conn_left = np.asarray(conn_left).astype(np.int32)
    conn_right = np.asarray(conn_right).astype(np.int32)
    emb = np.ascontiguousarray(np.asarray(emb), dtype=np.float32)
    W_bil = np.asarray(W_bil, dtype=np.float32)
    W_tailT = np.ascontiguousarray(np.asarray(W_tail, dtype=np.float32).T)
    W_headT = np.ascontiguousarray(np.asarray(W_head, dtype=np.float32).T)
    gamma_b = np.ascontiguousarray(
        np.broadcast_to(np.asarray(gamma, np.float32), (128, D)))
    beta_b = np.ascontiguousarray(
        np.broadcast_to(np.asarray(beta, np.float32), (128, D)))

    in_maps = []
    for c in range(N_CORES):
        sl = slice(c * B, (c + 1) * B)
        ent = entity[sl]                       # [128, 2]
        m = {
            "W_bil": W_bil, "W_tailT": W_tailT, "W_headT": W_headT,
            "gamma_b": gamma_b, "beta_b": beta_b,
        }
        # heads
        uniq_h, inv_h = np.unique(ent, return_inverse=True)
        head_table = np.zeros((UH_MAX, D), np.float32)
        head_table[: uniq_h.shape[0]] = emb[uniq_h]
        m["head_table"] = head_table
        m["ent_idx"] = inv_h.reshape(128, 2).astype(np.int32)

        for side, conn in (("L", conn_left), ("R", conn_right)):
            ids = conn[sl]                     # [128, 200, 2]
            rel_ids, tail_ids = ids[..., 0], ids[..., 1]

            uniq_r, inv_r = np.unique(rel_ids, return_inverse=True)
            inv_r = inv_r.reshape(B, M)
            rel_table = np.zeros((U_MAX, D), np.float32)
            rel_table[: uniq_r.shape[0]] = emb[uniq_r]
            m[f"rel_table_{side}"] = rel_table
            # m-grouped: position i = m*128 + b
            m[f"rel_idx_{side}"] = _wrap16(
                inv_r.T.reshape(-1).astype(np.int16))

            uniq_t, inv_t = np.unique(tail_ids, return_inverse=True)
            inv_t = inv_t.reshape(B, M)
            ut = uniq_t.shape[0]
            tail_table = np.zeros((U_MAX, D), np.float32)
            tail_table[:ut] = emb[uniq_t]
            m[f"tail_table_{side}"] = tail_table
            # b-grouped padded: X[b, c, p] = inv_t[b, c*128+p] (pad -> ut)
            X = np.full((B, 2, 128), ut, np.int16)
            X[:, 0, :] = inv_t[:, 0:128]
            X[:, 1, :72] = inv_t[:, 128:200]
            m[f"tail_idx_{side}"] = _wrap16(X.reshape(-1))

            m[f"pen_{side}"] = np.where(
                rel_ids == PAD_IDX, -1e30, 0.0).astype(np.float32)
        in_maps.append(m)
    return in_maps


def _get_program(repeat: int = 1):
    key = ("nc", repeat)
    if key not in _PROGRAM_CACHE:
        _PROGRAM_CACHE[key] = _build_program(repeat)
    return _PROGRAM_CACHE[key]


def kernel(entity, conn_left, conn_right, emb, W_bil, W_tail, W_head,
           gamma, beta):
    nc = _get_program()
    in_maps = _prep_inputs(entity, conn_left, conn_right, emb, W_bil, W_tail,
                           W_head, gamma, beta)
    res = run_bass_kernel_spmd(nc, in_maps, core_ids=list(range(N_CORES)))
    left = np.concatenate([np.asarray(r["out_L"]) for r in res.results], axis=0)
    right = np.concatenate([np.asarray(r["out_R"]) for r in res.results], axis=0)
    return left, right


# revision 9
# speedup vs baseline: 37.5897x; 1.2867x over previous
"""Trainium2 Bass kernel for nn_EntityEncoder (gnn_message_passing).

Full inputs in, full outputs out. Internally: data-parallel over batch across
8 NeuronCores (128 batch rows per core). Embedding rows are fetched on-device
via dma_gather from per-core compacted tables (int16 index constraint);
attention scores via fused DVE dot-products; attention-apply via per-batch
stationary matmuls on the tensor engine; residual + LayerNorm fused on
DVE/ACT.
"""

import numpy as np

import concourse.tile_sem_assignment as _tsa

# Walrus rejects instructions carrying >2 semaphore waits and Tile's
# FIFO-dominance wait elision is disabled; a single SWDGE completion lane
# keeps every instruction's wait count within the ISA limit.
_tsa.NUM_SWDGE_GLOBAL_SEMS = 1

from concourse import bacc, bass, mybir  # noqa: E402
import concourse.tile as tile  # noqa: E402
from concourse.bass_utils import run_bass_kernel_spmd  # noqa: E402
from concourse.masks import make_identity  # noqa: E402

# Problem constants (hardcoded per harness contract).
D = 128            # embed dim
B_FULL = 1024      # full batch
M = 200            # max neighbors
N_CORES = 8
B = B_FULL // N_CORES  # 128 rows per core
PAD_IDX = 100000
LN_EPS = 1e-5

# Compact-table sizing: per side <=128*200 unique rel/tail ids, +1 zero row.
U_MAX = 25728      # fixed table row count (>= 25601), identical across cores
UH_MAX = 257       # head table rows (<=256 unique entity ids + zero row)

REL_CHUNK_COLS = 50   # m-values per rel gather  -> 6400 rows/instr
TAIL_CHUNK_COLS = 32  # tile columns per tail gather -> 4096 rows/instr

_F32 = mybir.dt.float32
_I16 = mybir.dt.int16
_I32 = mybir.dt.int32
_AX = mybir.AxisListType
_OP = mybir.AluOpType
_ACT = mybir.ActivationFunctionType

_PROGRAM_CACHE = {}


def _wrap16(ids16: np.ndarray) -> np.ndarray:
    """Flat int16 index list -> [128, N/16] wrapped/replicated dma_gather layout."""
    n = ids16.shape[0]
    assert n % 16 == 0
    blk = ids16.reshape(n // 16, 16).T  # [16, n/16]
    return np.tile(blk, (8, 1)).astype(np.int16)


def _build_side(nc, tc, consts, side, ios):
    """Emit one side's (left/right) compute. consts holds shared tiles."""
    sb = consts["sb"]
    relbuf = consts["relbuf"]
    tailbuf = consts["tailbuf"]
    psum = consts["psum"]
    u_s = consts[f"u_{side}"]
    headT_s = consts[f"headT_{side}"]
    head_nat_s = consts[f"head_nat_{side}"]

    rel_table = ios[f"rel_table_{side}"]
    tail_table = ios[f"tail_table_{side}"]
    rel_idx_d = ios[f"rel_idx_{side}"]
    tail_idx_d = ios[f"tail_idx_{side}"]
    pen_d = ios[f"pen_{side}"]
    out_d = ios[f"out_{side}"]

    # --- small loads -------------------------------------------------------
    rel_idx = sb.tile([128, (M * 128) // 16], _I16, tag=f"rel_idx_{side}")
    nc.gpsimd.dma_start(out=rel_idx[:], in_=rel_idx_d[:])
    tail_idx = sb.tile([128, (M * 128) // 16], _I16, tag=f"tail_idx_{side}")
    nc.gpsimd.dma_start(out=tail_idx[:], in_=tail_idx_d[:])
    pen = sb.tile([128, M], _F32, tag=f"pen_{side}")
    nc.gpsimd.dma_start(out=pen[:], in_=pen_d[:])

    # --- scores: score[b, m] = u[b, :] . rel[b, m, :]  ---------------------
    score = sb.tile([128, M], _F32, tag=f"score_{side}")
    for mc in range(0, M, REL_CHUNK_COLS):
        k = min(REL_CHUNK_COLS, M - mc)
        rel_chunk = relbuf.tile([128, k, D], _F32, tag="rel_chunk")
        nc.gpsimd.dma_gather(
            rel_chunk[:],
            rel_table[:],
            rel_idx[:, (mc * 128) // 16 : ((mc + k) * 128) // 16],
            k * 128,
            k * 128,
            D,
            single_packet=False,
        )
        for j in range(k):
            scratch = consts["scratch"].tile([128, D], _F32, tag="dot_scratch")
            nc.vector.scalar_tensor_tensor(
                out=scratch[:],
                in0=rel_chunk[:, j, :],
                scalar=1.0,
                in1=u_s[:],
                op0=_OP.mult,
                op1=_OP.mult,
                accum_out=score[:, mc + j : mc + j + 1],
            )

    # mask penalty (pad neighbors -> -1e30)
    nc.vector.tensor_tensor(out=score[:], in0=score[:], in1=pen[:], op=_OP.add)

    # --- softmax over m ----------------------------------------------------
    rmax = sb.tile([128, 1], _F32, tag=f"rmax_{side}")
    nc.vector.reduce_max(rmax[:], score[:], axis=_AX.X)
    negmax = sb.tile([128, 1], _F32, tag=f"negmax_{side}")
    nc.vector.tensor_scalar_mul(negmax[:], rmax[:], -1.0)
    expt = sb.tile([128, M], _F32, tag=f"expt_{side}")
    zsum = sb.tile([128, 1], _F32, tag=f"zsum_{side}")
    nc.scalar.activation(
        out=expt[:], in_=score[:], func=_ACT.Exp,
        bias=negmax[:, :1], scale=1.0, accum_out=zsum[:],
    )
    rz = sb.tile([128, 1], _F32, tag=f"rz_{side}")
    nc.vector.reciprocal(rz[:], zsum[:])
    att = sb.tile([128, M], _F32, tag=f"att_{side}")
    nc.vector.tensor_scalar_mul(att[:], expt[:], rz[:, :1])

    # --- transpose att[:, :128] -> [m, b] columns for per-b matmul rhs -----
    attT0_p = psum.tile([128, 128], _F32, space="PSUM", tag="tr_p")
    nc.tensor.transpose(out=attT0_p[:], in_=att[:, 0:128], identity=consts["ident"][:])
    attT0 = sb.tile([128, 128], _F32, tag=f"attT0_{side}")
    nc.vector.tensor_copy(out=attT0[:], in_=attT0_p[:])

    # --- attention-apply, m 0..127 (b-grouped): aggT[:, b] via PE ----------
    aggT_p = psum.tile([128, 128], _F32, space="PSUM", tag="aggT_p")
    for tc0 in range(0, 128, TAIL_CHUNK_COLS):
        k = TAIL_CHUNK_COLS
        tail_chunk = tailbuf.tile([128, k, D], _F32, tag="tail_chunk")
        nc.gpsimd.dma_gather(
            tail_chunk[:],
            tail_table[:],
            tail_idx[:, (tc0 * 128) // 16 : ((tc0 + k) * 128) // 16],
            k * 128,
            k * 128,
            D,
            single_packet=False,
        )
        for j in range(k):
            b = tc0 + j
            nc.tensor.matmul(
                out=aggT_p[:, b : b + 1],
                lhsT=tail_chunk[:, j, :],
                rhs=attT0[:, b : b + 1],
                start=True, stop=True,
            )
    aggT = sb.tile([128, 128], _F32, tag=f"aggT_{side}")
    nc.vector.tensor_copy(out=aggT[:], in_=aggT_p[:])

    # --- attention-apply, m 128..199 (m-grouped): DVE MAC accumulate -------
    agg1 = sb.tile([128, D], _F32, tag=f"agg1_{side}")
    nc.vector.memset(agg1[:], 0.0)
    for mc in range(128, 200, 36):
        k = min(36, 200 - mc)
        t1_chunk = tailbuf.tile([128, k, D], _F32, tag="t1_chunk")
        nc.gpsimd.dma_gather(
            t1_chunk[:],
            tail_table[:],
            tail_idx[:, ((mc) * 128) // 16 : ((mc + k) * 128) // 16],
            k * 128,
            k * 128,
            D,
            single_packet=False,
        )
        for j in range(k):
            m = mc + j
            nc.vector.scalar_tensor_tensor(
                out=agg1[:],
                in0=t1_chunk[:, j, :],
                scalar=att[:, m : m + 1],
                in1=agg1[:],
                op0=_OP.mult,
                op1=_OP.add,
            )
    # fold agg1 (natural [b, d]) into aggT: transpose then add
    agg1T_p = psum.tile([128, 128], _F32, space="PSUM", tag="tr_p")
    nc.tensor.transpose(out=agg1T_p[:], in_=agg1[:], identity=consts["ident"][:])
    nc.vector.tensor_tensor(out=aggT[:], in0=aggT[:], in1=agg1T_p[:], op=_OP.add)

    # --- branch: h = relu(agg @ Wt^T + head @ Wh^T);  x = h + head; LN -----
    h_p = consts["psum1"].tile([128, 128], _F32, space="PSUM", tag="h_p")
    nc.tensor.matmul(out=h_p[:], lhsT=aggT[:], rhs=consts["W_tailT"][:],
                     start=True, stop=False)
    nc.tensor.matmul(out=h_p[:], lhsT=headT_s[:], rhs=consts["W_headT"][:],
                     start=False, stop=True)
    h = sb.tile([128, 128], _F32, tag=f"h_{side}")
    nc.scalar.activation(out=h[:], in_=h_p[:], func=_ACT.Relu)

    x = sb.tile([128, 128], _F32, tag=f"x_{side}")
    nc.vector.tensor_tensor(out=x[:], in0=h[:], in1=head_nat_s[:], op=_OP.add)

    s1 = sb.tile([128, 1], _F32, tag=f"s1_{side}")
    nc.vector.reduce_sum(s1[:], x[:], axis=_AX.X)
    negmu = sb.tile([128, 1], _F32, tag=f"negmu_{side}")
    nc.vector.tensor_scalar_mul(negmu[:], s1[:], -1.0 / D)
    xc = sb.tile([128, 128], _F32, tag=f"xc_{side}")
    nc.scalar.activation(out=xc[:], in_=x[:], func=_ACT.Identity, bias=negmu[:, :1])
    sq = sb.tile([128, 128], _F32, tag=f"sq_{side}")
    ssq = sb.tile([128, 1], _F32, tag=f"ssq_{side}")
    nc.scalar.activation(out=sq[:], in_=xc[:], func=_ACT.Square, accum_out=ssq[:])
    std = sb.tile([128, 1], _F32, tag=f"std_{side}")
    # std = sqrt(var + eps) = sqrt(ssq/D + eps)
    nc.scalar.activation(out=std[:], in_=ssq[:], func=_ACT.Sqrt,
                         bias=consts["eps"][:, :1], scale=1.0 / D)
    rstd = sb.tile([128, 1], _F32, tag=f"rstd_{side}")
    nc.vector.reciprocal(rstd[:], std[:])

    y = sb.tile([128, 128], _F32, tag=f"y_{side}")
    nc.vector.scalar_tensor_tensor(
        out=y[:], in0=xc[:], scalar=rstd[:, :1], in1=consts["gamma_b"][:],
        op0=_OP.mult, op1=_OP.mult,
    )
    yb = sb.tile([128, 128], _F32, tag=f"yb_{side}")
    nc.vector.tensor_tensor(out=yb[:], in0=y[:], in1=consts["beta_b"][:], op=_OP.add)
    nc.gpsimd.dma_start(out=out_d[:], in_=yb[:])


def _build_program(repeat: int = 1):
    nc = bacc.Bacc(None, target_bir_lowering=False, debug=False)

    ios = {}
    for side in ("L", "R"):
        ios[f"rel_table_{side}"] = nc.declare_dram_parameter(
            f"rel_table_{side}", [U_MAX, D], _F32, isOutput=False)
        ios[f"tail_table_{side}"] = nc.declare_dram_parameter(
            f"tail_table_{side}", [U_MAX, D], _F32, isOutput=False)
        ios[f"rel_idx_{side}"] = nc.declare_dram_parameter(
            f"rel_idx_{side}", [128, (M * 128) // 16], _I16, isOutput=False)
        ios[f"tail_idx_{side}"] = nc.declare_dram_parameter(
            f"tail_idx_{side}", [128, (M * 128) // 16], _I16, isOutput=False)
        ios[f"pen_{side}"] = nc.declare_dram_parameter(
            f"pen_{side}", [128, M], _F32, isOutput=False)
        ios[f"out_{side}"] = nc.declare_dram_parameter(
            f"out_{side}", [128, D], _F32, isOutput=True)
    ios["head_table"] = nc.declare_dram_parameter(
        "head_table", [UH_MAX, D], _F32, isOutput=False)
    ios["ent_idx"] = nc.declare_dram_parameter(
        "ent_idx", [128, 2], _I32, isOutput=False)
    for w in ("W_bil", "W_tailT", "W_headT", "gamma_b", "beta_b"):
        ios[w] = nc.declare_dram_parameter(w, [128, 128], _F32, isOutput=False)

    with tile.TileContext(nc) as tc:
        with (
            tc.tile_pool(name="sb", bufs=1) as sb,
            tc.tile_pool(name="relbuf", bufs=2) as relbuf,
            tc.tile_pool(name="tailbuf", bufs=2) as tailbuf,
            tc.tile_pool(name="scratch", bufs=4) as scratch,
            tc.tile_pool(name="psum", bufs=2, space="PSUM") as psum,
            tc.tile_pool(name="psum1", bufs=1, space="PSUM") as psum1,
        ):
            consts = {
                "sb": sb, "relbuf": relbuf, "tailbuf": tailbuf,
                "scratch": scratch, "psum": psum, "psum1": psum1,
            }
            # constants
            for w in ("W_bil", "W_tailT", "W_headT", "gamma_b", "beta_b"):
                t = sb.tile([128, 128], _F32, tag=w)
                nc.gpsimd.dma_start(out=t[:], in_=ios[w][:])
                consts[w] = t
            ident = sb.tile([128, 128], _F32, tag="ident")
            make_identity(nc, ident[:])
            consts["ident"] = ident
            eps = sb.tile([128, 1], _F32, tag="eps")
            nc.vector.memset(eps[:], LN_EPS)
            consts["eps"] = eps

            def body():
                # heads: gather, transpose, u = (headR - headL) @ W_bil
                ent_idx = sb.tile([128, 2], _I32, tag="ent_idx")
                nc.gpsimd.dma_start(out=ent_idx[:], in_=ios["ent_idx"][:])
                headT = {}
                for i, side in enumerate(("L", "R")):
                    hn = sb.tile([128, D], _F32, tag=f"head_nat_{side}")
                    nc.gpsimd.indirect_dma_start(
                        out=hn[:], out_offset=None, in_=ios["head_table"][:],
                        in_offset=bass.IndirectOffsetOnAxis(
                            ap=ent_idx[:, i : i + 1], axis=0),
                    )
                    consts[f"head_nat_{side}"] = hn
                    hT_p = psum.tile([128, 128], _F32, space="PSUM", tag="tr_p")
                    nc.tensor.transpose(out=hT_p[:], in_=hn[:], identity=ident[:])
                    hT = sb.tile([128, 128], _F32, tag=f"headT_{side}")
                    nc.vector.tensor_copy(out=hT[:], in_=hT_p[:])
                    headT[side] = hT
                    consts[f"headT_{side}"] = hT

                wrT = sb.tile([128, 128], _F32, tag="wrT")
                nc.vector.tensor_tensor(
                    out=wrT[:], in0=headT["R"][:], in1=headT["L"][:],
                    op=_OP.subtract)
                # u[b, e] = sum_d wrT[d, b] * W_bil[d, e]   (for both sides)
                u_p = psum1.tile([128, 128], _F32, space="PSUM", tag="u_p")
                nc.tensor.matmul(out=u_p[:], lhsT=wrT[:], rhs=consts["W_bil"][:],
                                 start=True, stop=True)
                u = sb.tile([128, 128], _F32, tag="u")
                nc.vector.tensor_copy(out=u[:], in_=u_p[:])
                consts["u_L"] = u
                consts["u_R"] = u

                for side in ("L", "R"):
                    _build_side(nc, tc, consts, side, ios)

            if repeat == 1:
                body()
            else:
                with tc.For_i(0, repeat, 1):
                    body()

    nc.finalize()
    return nc


def _prep_inputs(entity, conn_left, conn_right, emb, W_bil, W_tail, W_head,
                 gamma, beta):
    """Host-side sharding + compaction. Returns per-core input maps."""
    entity = np.asarray(entity).astype(np.int32)
    conn_left = np.asarray(conn_left).astype(np.int32)
    conn_right = np.asarray(conn_right).astype(np.int32)
    emb = np.ascontiguousarray(np.asarray(emb), dtype=np.float32)
    W_bil = np.asarray(W_bil, dtype=np.float32)
    W_tailT = np.ascontiguousarray(np.asarray(W_tail, dtype=np.float32).T)
    W_headT = np.ascontiguousarray(np.asarray(W_head, dtype=np.float32).T)
    gamma_b = np.ascontiguousarray(
        np.broadcast_to(np.asarray(gamma, np.float32), (128, D)))
    beta_b = np.ascontiguousarray(
        np.broadcast_to(np.asarray(beta, np.float32), (128, D)))

    in_maps = []
    for c in range(N_CORES):
        sl = slice(c * B, (c + 1) * B)
        ent = entity[sl]                       # [128, 2]
        m = {
            "W_bil": W_bil, "W_tailT": W_tailT, "W_headT": W_headT,
            "gamma_b": gamma_b, "beta_b": beta_b,
        }
        # heads
        uniq_h, inv_h = np.unique(ent, return_inverse=True)
        head_table = np.zeros((UH_MAX, D), np.float32)
        head_table[: uniq_h.shape[0]] = emb[uniq_h]
        m["head_table"] = head_table
        m["ent_idx"] = inv_h.reshape(128, 2).astype(np.int32)

        for side, conn in (("L", conn_left), ("R", conn_right)):
            ids = conn[sl]                     # [128, 200, 2]
            rel_ids, tail_ids = ids[..., 0], ids[..., 1]

            uniq_r, inv_r = np.unique(rel_ids, return_inverse=True)
            inv_r = inv_r.reshape(B, M)
            rel_table = np.zeros((U_MAX, D), np.float32)
            rel_table[: uniq_r.shape[0]] = emb[uniq_r]
            m[f"rel_table_{side}"] = rel_table
            # m-grouped: position i = m*128 + b
            m[f"rel_idx_{side}"] = _wrap16(
                inv_r.T.reshape(-1).astype(np.int16))

            uniq_t, inv_t = np.unique(tail_ids, return_inverse=True)
            inv_t = inv_t.reshape(B, M)
            tail_table = np.zeros((U_MAX, D), np.float32)
            tail_table[: uniq_t.shape[0]] = emb[uniq_t]
            m[f"tail_table_{side}"] = tail_table
            # cols 0..127 b-grouped (m 0..127); cols 128..199 m-grouped
            part0 = inv_t[:, 0:128].reshape(-1)
            part1 = inv_t[:, 128:200].T.reshape(-1)
            m[f"tail_idx_{side}"] = _wrap16(
                np.concatenate([part0, part1]).astype(np.int16))

            m[f"pen_{side}"] = np.where(
                rel_ids == PAD_IDX, -1e30, 0.0).astype(np.float32)
        in_maps.append(m)
    return in_maps


def _get_program(repeat: int = 1):
    key = ("nc", repeat)
    if key not in _PROGRAM_CACHE:
        _PROGRAM_CACHE[key] = _build_program(repeat)
    return _PROGRAM_CACHE[key]


def kernel(entity, conn_left, conn_right, emb, W_bil, W_tail, W_head,
           gamma, beta):
    nc = _get_program()
    in_maps = _prep_inputs(entity, conn_left, conn_right, emb, W_bil, W_tail,
                           W_head, gamma, beta)
    res = run_bass_kernel_spmd(nc, in_maps, core_ids=list(range(N_CORES)))
    left = np.concatenate([np.asarray(r["out_L"]) for r in res.results], axis=0)
    right = np.concatenate([np.asarray(r["out_R"]) for r in res.results], axis=0)
    return left, right


# revision 10
# speedup vs baseline: 37.7322x; 1.0038x over previous
"""Trainium2 Bass kernel for nn_EntityEncoder (gnn_message_passing).

Full inputs in, full outputs out. Internally: data-parallel over batch across
8 NeuronCores (128 batch rows per core). Embedding rows are fetched on-device
via dma_gather from per-core compacted tables (int16 index constraint);
attention scores via fused DVE dot-products; attention-apply via per-batch
stationary matmuls on the tensor engine; residual + LayerNorm fused on
DVE/ACT.
"""

import numpy as np

import concourse.tile_sem_assignment as _tsa

# Walrus rejects instructions carrying >2 semaphore waits and Tile's
# FIFO-dominance wait elision is disabled; a single SWDGE completion lane
# keeps every instruction's wait count within the ISA limit.
_tsa.NUM_SWDGE_GLOBAL_SEMS = 1

from concourse import bacc, bass, mybir  # noqa: E402
import concourse.tile as tile  # noqa: E402
from concourse.bass_utils import run_bass_kernel_spmd  # noqa: E402
from concourse.masks import make_identity  # noqa: E402

# Problem constants (hardcoded per harness contract).
D = 128            # embed dim
B_FULL = 1024      # full batch
M = 200            # max neighbors
N_CORES = 8
B = B_FULL // N_CORES  # 128 rows per core
PAD_IDX = 100000
LN_EPS = 1e-5

# Compact-table sizing: per side <=128*200 unique rel/tail ids, +1 zero row.
U_MAX = 25728      # fixed table row count (>= 25601), identical across cores
UH_MAX = 257       # head table rows (<=256 unique entity ids + zero row)

REL_CHUNK_COLS = 50   # m-values per rel gather  -> 6400 rows/instr
TAIL_CHUNK_COLS = 32  # tile columns per tail gather -> 4096 rows/instr

_F32 = mybir.dt.float32
_I16 = mybir.dt.int16
_I32 = mybir.dt.int32
_AX = mybir.AxisListType
_OP = mybir.AluOpType
_ACT = mybir.ActivationFunctionType

_PROGRAM_CACHE = {}


def _wrap16(ids16: np.ndarray) -> np.ndarray:
    """Flat int16 index list -> [128, N/16] wrapped/replicated dma_gather layout."""
    n = ids16.shape[0]
    assert n % 16 == 0
    blk = ids16.reshape(n // 16, 16).T  # [16, n/16]
    return np.tile(blk, (8, 1)).astype(np.int16)


def _build_side(nc, tc, consts, side, ios):
    """Emit one side's (left/right) compute. consts holds shared tiles."""
    sb = consts["sb"]
    relbuf = consts["relbuf"]
    tailbuf = consts["tailbuf"]
    psum = consts["psum"]
    u_s = consts[f"u_{side}"]
    headT_s = consts[f"headT_{side}"]
    head_nat_s = consts[f"head_nat_{side}"]

    rel_table = ios[f"rel_table_{side}"]
    tail_table = ios[f"tail_table_{side}"]
    rel_idx_d = ios[f"rel_idx_{side}"]
    tail_idx_d = ios[f"tail_idx_{side}"]
    pen_d = ios[f"pen_{side}"]
    out_d = ios[f"out_{side}"]

    # --- small loads -------------------------------------------------------
    rel_idx = sb.tile([128, (M * 128) // 16], _I16, tag=f"rel_idx_{side}")
    nc.gpsimd.dma_start(out=rel_idx[:], in_=rel_idx_d[:])
    tail_idx = sb.tile([128, (M * 128) // 16], _I16, tag=f"tail_idx_{side}")
    nc.gpsimd.dma_start(out=tail_idx[:], in_=tail_idx_d[:])
    pen = sb.tile([128, M], _F32, tag=f"pen_{side}")
    nc.gpsimd.dma_start(out=pen[:], in_=pen_d[:])

    # --- scores: score[b, m] = u[b, :] . rel[b, m, :]  ---------------------
    score = sb.tile([128, M], _F32, tag=f"score_{side}")
    for mc in range(0, M, REL_CHUNK_COLS):
        k = min(REL_CHUNK_COLS, M - mc)
        rel_chunk = relbuf.tile([128, k, D], _F32, tag="rel_chunk")
        nc.gpsimd.dma_gather(
            rel_chunk[:],
            rel_table[:],
            rel_idx[:, (mc * 128) // 16 : ((mc + k) * 128) // 16],
            k * 128,
            k * 128,
            D,
            single_packet=False,
        )
        for j in range(k):
            scratch = consts["scratch"].tile([128, D], _F32, tag="dot_scratch")
            nc.vector.scalar_tensor_tensor(
                out=scratch[:],
                in0=rel_chunk[:, j, :],
                scalar=1.0,
                in1=u_s[:],
                op0=_OP.mult,
                op1=_OP.mult,
                accum_out=score[:, mc + j : mc + j + 1],
            )

    # mask penalty (pad neighbors -> -1e30)
    nc.vector.tensor_tensor(out=score[:], in0=score[:], in1=pen[:], op=_OP.add)

    # --- softmax over m ----------------------------------------------------
    rmax = sb.tile([128, 1], _F32, tag=f"rmax_{side}")
    nc.vector.reduce_max(rmax[:], score[:], axis=_AX.X)
    negmax = sb.tile([128, 1], _F32, tag=f"negmax_{side}")
    nc.vector.tensor_scalar_mul(negmax[:], rmax[:], -1.0)
    expt = sb.tile([128, M], _F32, tag=f"expt_{side}")
    zsum = sb.tile([128, 1], _F32, tag=f"zsum_{side}")
    nc.scalar.activation(
        out=expt[:], in_=score[:], func=_ACT.Exp,
        bias=negmax[:, :1], scale=1.0, accum_out=zsum[:],
    )
    rz = sb.tile([128, 1], _F32, tag=f"rz_{side}")
    nc.vector.reciprocal(rz[:], zsum[:])
    att = sb.tile([128, M], _F32, tag=f"att_{side}")
    nc.vector.tensor_scalar_mul(att[:], expt[:], rz[:, :1])

    # --- transpose att[:, :128] -> [m, b] columns for per-b matmul rhs -----
    attT0_p = psum.tile([128, 128], _F32, space="PSUM", tag="tr_p")
    nc.tensor.transpose(out=attT0_p[:], in_=att[:, 0:128], identity=consts["ident"][:])
    attT0 = sb.tile([128, 128], _F32, tag=f"attT0_{side}")
    nc.scalar.copy(out=attT0[:], in_=attT0_p[:])

    # --- attention-apply, m 0..127 (b-grouped): aggT[:, b] via PE ----------
    aggT_p = psum.tile([128, 128], _F32, space="PSUM", tag="aggT_p")
    for tc0 in range(0, 128, TAIL_CHUNK_COLS):
        k = TAIL_CHUNK_COLS
        tail_chunk = tailbuf.tile([128, k, D], _F32, tag="tail_chunk")
        nc.gpsimd.dma_gather(
            tail_chunk[:],
            tail_table[:],
            tail_idx[:, (tc0 * 128) // 16 : ((tc0 + k) * 128) // 16],
            k * 128,
            k * 128,
            D,
            single_packet=False,
        )
        for j in range(k):
            b = tc0 + j
            nc.tensor.matmul(
                out=aggT_p[:, b : b + 1],
                lhsT=tail_chunk[:, j, :],
                rhs=attT0[:, b : b + 1],
                start=True, stop=True,
            )
    aggT = sb.tile([128, 128], _F32, tag=f"aggT_{side}")
    nc.scalar.copy(out=aggT[:], in_=aggT_p[:])

    # --- attention-apply, m 128..199 (m-grouped): DVE MAC accumulate -------
    agg1 = sb.tile([128, D], _F32, tag=f"agg1_{side}")
    nc.vector.memset(agg1[:], 0.0)
    for mc in range(128, 200, 36):
        k = min(36, 200 - mc)
        t1_chunk = tailbuf.tile([128, k, D], _F32, tag="t1_chunk")
        nc.gpsimd.dma_gather(
            t1_chunk[:],
            tail_table[:],
            tail_idx[:, ((mc) * 128) // 16 : ((mc + k) * 128) // 16],
            k * 128,
            k * 128,
            D,
            single_packet=False,
        )
        for j in range(k):
            m = mc + j
            nc.vector.scalar_tensor_tensor(
                out=agg1[:],
                in0=t1_chunk[:, j, :],
                scalar=att[:, m : m + 1],
                in1=agg1[:],
                op0=_OP.mult,
                op1=_OP.add,
            )
    # fold agg1 (natural [b, d]) into aggT: transpose then add
    agg1T_p = psum.tile([128, 128], _F32, space="PSUM", tag="tr_p")
    nc.tensor.transpose(out=agg1T_p[:], in_=agg1[:], identity=consts["ident"][:])
    nc.vector.tensor_tensor(out=aggT[:], in0=aggT[:], in1=agg1T_p[:], op=_OP.add)

    # --- branch: h = relu(agg @ Wt^T + head @ Wh^T);  x = h + head; LN -----
    h_p = consts["psum1"].tile([128, 128], _F32, space="PSUM", tag="h_p")
    nc.tensor.matmul(out=h_p[:], lhsT=aggT[:], rhs=consts["W_tailT"][:],
                     start=True, stop=False)
    nc.tensor.matmul(out=h_p[:], lhsT=headT_s[:], rhs=consts["W_headT"][:],
                     start=False, stop=True)
    h = sb.tile([128, 128], _F32, tag=f"h_{side}")
    nc.scalar.activation(out=h[:], in_=h_p[:], func=_ACT.Relu)

    x = sb.tile([128, 128], _F32, tag=f"x_{side}")
    nc.vector.tensor_tensor(out=x[:], in0=h[:], in1=head_nat_s[:], op=_OP.add)

    s1 = sb.tile([128, 1], _F32, tag=f"s1_{side}")
    nc.vector.reduce_sum(s1[:], x[:], axis=_AX.X)
    negmu = sb.tile([128, 1], _F32, tag=f"negmu_{side}")
    nc.vector.tensor_scalar_mul(negmu[:], s1[:], -1.0 / D)
    xc = sb.tile([128, 128], _F32, tag=f"xc_{side}")
    nc.scalar.activation(out=xc[:], in_=x[:], func=_ACT.Identity, bias=negmu[:, :1])
    sq = sb.tile([128, 128], _F32, tag=f"sq_{side}")
    ssq = sb.tile([128, 1], _F32, tag=f"ssq_{side}")
    nc.scalar.activation(out=sq[:], in_=xc[:], func=_ACT.Square, accum_out=ssq[:])
    std = sb.tile([128, 1], _F32, tag=f"std_{side}")
    # std = sqrt(var + eps) = sqrt(ssq/D + eps)
    nc.scalar.activation(out=std[:], in_=ssq[:], func=_ACT.Sqrt,
                         bias=consts["eps"][:, :1], scale=1.0 / D)
    rstd = sb.tile([128, 1], _F32, tag=f"rstd_{side}")
    nc.vector.reciprocal(rstd[:], std[:])

    y = sb.tile([128, 128], _F32, tag=f"y_{side}")
    nc.vector.scalar_tensor_tensor(
        out=y[:], in0=xc[:], scalar=rstd[:, :1], in1=consts["gamma_b"][:],
        op0=_OP.mult, op1=_OP.mult,
    )
    yb = sb.tile([128, 128], _F32, tag=f"yb_{side}")
    nc.vector.tensor_tensor(out=yb[:], in0=y[:], in1=consts["beta_b"][:], op=_OP.add)
    nc.gpsimd.dma_start(out=out_d[:], in_=yb[:])


def _build_program(repeat: int = 1):
    nc = bacc.Bacc(None, target_bir_lowering=False, debug=False)

    ios = {}
    for side in ("L", "R"):
        ios[f"rel_table_{side}"] = nc.declare_dram_parameter(
            f"rel_table_{side}", [U_MAX, D], _F32, isOutput=False)
        ios[f"tail_table_{side}"] = nc.declare_dram_parameter(
            f"tail_table_{side}", [U_MAX, D], _F32, isOutput=False)
        ios[f"rel_idx_{side}"] = nc.declare_dram_parameter(
            f"rel_idx_{side}", [128, (M * 128) // 16], _I16, isOutput=False)
        ios[f"tail_idx_{side}"] = nc.declare_dram_parameter(
            f"tail_idx_{side}", [128, (M * 128) // 16], _I16, isOutput=False)
        ios[f"pen_{side}"] = nc.declare_dram_parameter(
            f"pen_{side}", [128, M], _F32, isOutput=False)
        ios[f"out_{side}"] = nc.declare_dram_parameter(
            f"out_{side}", [128, D], _F32, isOutput=True)
    ios["head_table"] = nc.declare_dram_parameter(
        "head_table", [UH_MAX, D], _F32, isOutput=False)
    ios["ent_idx"] = nc.declare_dram_parameter(
        "ent_idx", [128, 2], _I32, isOutput=False)
    for w in ("W_bil", "W_tailT", "W_headT", "gamma_b", "beta_b"):
        ios[w] = nc.declare_dram_parameter(w, [128, 128], _F32, isOutput=False)

    with tile.TileContext(nc) as tc:
        with (
            tc.tile_pool(name="sb", bufs=1) as sb,
            tc.tile_pool(name="relbuf", bufs=3) as relbuf,
            tc.tile_pool(name="tailbuf", bufs=2) as tailbuf,
            tc.tile_pool(name="scratch", bufs=6) as scratch,
            tc.tile_pool(name="psum", bufs=2, space="PSUM") as psum,
            tc.tile_pool(name="psum1", bufs=1, space="PSUM") as psum1,
        ):
            consts = {
                "sb": sb, "relbuf": relbuf, "tailbuf": tailbuf,
                "scratch": scratch, "psum": psum, "psum1": psum1,
            }
            # constants
            for w in ("W_bil", "W_tailT", "W_headT", "gamma_b", "beta_b"):
                t = sb.tile([128, 128], _F32, tag=w)
                nc.gpsimd.dma_start(out=t[:], in_=ios[w][:])
                consts[w] = t
            ident = sb.tile([128, 128], _F32, tag="ident")
            make_identity(nc, ident[:])
            consts["ident"] = ident
            eps = sb.tile([128, 1], _F32, tag="eps")
            nc.vector.memset(eps[:], LN_EPS)
            consts["eps"] = eps

            def body():
                # heads: gather, transpose, u = (headR - headL) @ W_bil
                ent_idx = sb.tile([128, 2], _I32, tag="ent_idx")
                nc.gpsimd.dma_start(out=ent_idx[:], in_=ios["ent_idx"][:])
                headT = {}
                for i, side in enumerate(("L", "R")):
                    hn = sb.tile([128, D], _F32, tag=f"head_nat_{side}")
                    nc.gpsimd.indirect_dma_start(
                        out=hn[:], out_offset=None, in_=ios["head_table"][:],
                        in_offset=bass.IndirectOffsetOnAxis(
                            ap=ent_idx[:, i : i + 1], axis=0),
                    )
                    consts[f"head_nat_{side}"] = hn
                    hT_p = psum.tile([128, 128], _F32, space="PSUM", tag="tr_p")
                    nc.tensor.transpose(out=hT_p[:], in_=hn[:], identity=ident[:])
                    hT = sb.tile([128, 128], _F32, tag=f"headT_{side}")
                    nc.vector.tensor_copy(out=hT[:], in_=hT_p[:])
                    headT[side] = hT
                    consts[f"headT_{side}"] = hT

                wrT = sb.tile([128, 128], _F32, tag="wrT")
                nc.vector.tensor_tensor(
                    out=wrT[:], in0=headT["R"][:], in1=headT["L"][:],
                    op=_OP.subtract)
                # u[b, e] = sum_d wrT[d, b] * W_bil[d, e]   (for both sides)
                u_p = psum1.tile([128, 128], _F32, space="PSUM", tag="u_p")
                nc.tensor.matmul(out=u_p[:], lhsT=wrT[:], rhs=consts["W_bil"][:],
                                 start=True, stop=True)
                u = sb.tile([128, 128], _F32, tag="u")
                nc.vector.tensor_copy(out=u[:], in_=u_p[:])
                consts["u_L"] = u
                consts["u_R"] = u

                for side in ("L", "R"):
                    _build_side(nc, tc, consts, side, ios)

            if repeat == 1:
                body()
            else:
                with tc.For_i(0, repeat, 1):
                    body()

    nc.finalize()
    return nc


def _prep_inputs(entity, conn_left, conn_right, emb, W_bil, W_tail, W_head,
                 gamma, beta):
    """Host-side sharding + compaction. Returns per-core input maps."""
    entity = np.asarray(entity).astype(np.int32)
    conn_left = np.asarray(conn_left).astype(np.int32)
    conn_right = np.asarray(conn_right).astype(np.int32)
    emb = np.ascontiguousarray(np.asarray(emb), dtype=np.float32)
    W_bil = np.asarray(W_bil, dtype=np.float32)
    W_tailT = np.ascontiguousarray(np.asarray(W_tail, dtype=np.float32).T)
    W_headT = np.ascontiguousarray(np.asarray(W_head, dtype=np.float32).T)
    gamma_b = np.ascontiguousarray(
        np.broadcast_to(np.asarray(gamma, np.float32), (128, D)))
    beta_b = np.ascontiguousarray(
        np.broadcast_to(np.asarray(beta, np.float32), (128, D)))

    in_maps = []
    for c in range(N_CORES):
        sl = slice(c * B, (c + 1) * B)
        ent = entity[sl]                       # [128, 2]
        m = {
            "W_bil": W_bil, "W_tailT": W_tailT, "W_headT": W_headT,
            "gamma_b": gamma_b, "beta_b": beta_b,
        }
        # heads
        uniq_h, inv_h = np.unique(ent, return_inverse=True)
        head_table = np.zeros((UH_MAX, D), np.float32)
        head_table[: uniq_h.shape[0]] = emb[uniq_h]
        m["head_table"] = head_table
        m["ent_idx"] = inv_h.reshape(128, 2).astype(np.int32)

        for side, conn in (("L", conn_left), ("R", conn_right)):
            ids = conn[sl]                     # [128, 200, 2]
            rel_ids, tail_ids = ids[..., 0], ids[..., 1]

            uniq_r, inv_r = np.unique(rel_ids, return_inverse=True)
            inv_r = inv_r.reshape(B, M)
            rel_table = np.zeros((U_MAX, D), np.float32)
            rel_table[: uniq_r.shape[0]] = emb[uniq_r]
            m[f"rel_table_{side}"] = rel_table
            # m-grouped: position i = m*128 + b
            m[f"rel_idx_{side}"] = _wrap16(
                inv_r.T.reshape(-1).astype(np.int16))

            uniq_t, inv_t = np.unique(tail_ids, return_inverse=True)
            inv_t = inv_t.reshape(B, M)
            tail_table = np.zeros((U_MAX, D), np.float32)
            tail_table[: uniq_t.shape[0]] = emb[uniq_t]
            m[f"tail_table_{side}"] = tail_table
            # cols 0..127 b-grouped (m 0..127); cols 128..199 m-grouped
            part0 = inv_t[:, 0:128].reshape(-1)
            part1 = inv_t[:, 128:200].T.reshape(-1)
            m[f"tail_idx_{side}"] = _wrap16(
                np.concatenate([part0, part1]).astype(np.int16))

            m[f"pen_{side}"] = np.where(
                rel_ids == PAD_IDX, -1e30, 0.0).astype(np.float32)
        in_maps.append(m)
    return in_maps


def _get_program(repeat: int = 1):
    key = ("nc", repeat)
    if key not in _PROGRAM_CACHE:
        _PROGRAM_CACHE[key] = _build_program(repeat)
    return _PROGRAM_CACHE[key]


def kernel(entity, conn_left, conn_right, emb, W_bil, W_tail, W_head,
           gamma, beta):
    nc = _get_program()
    in_maps = _prep_inputs(entity, conn_left, conn_right, emb, W_bil, W_tail,
                           W_head, gamma, beta)
    res = run_bass_kernel_spmd(nc, in_maps, core_ids=list(range(N_CORES)))
    left = np.concatenate([np.asarray(r["out_L"]) for r in res.results], axis=0)
    right = np.concatenate([np.asarray(r["out_R"]) for r in res.results], axis=0)
    return left, right
